# revision 1
# baseline (speedup 1.0000x reference)
"""Trainium2 Bass kernel for GroupQueryAttention (T=1024, D=2048, H=32, KV=8, HD=64).

Sharding: tensor-parallel over heads across 8 NeuronCores. Core r owns q-heads
4r..4r+3 and kv-head r (split Wq/Wk/Wv output dims). x is replicated. Instead
of the all-reduce-after-out_proj formulation, we AllGather the (small, 1MB)
per-core attention outputs and make the out-projection column-parallel: each
core computes a 256-column slice of the final [1024, 2048] output with the
full 2048-deep contraction. This moves 8x less data through the collective.

Per-core dataflow (all matmuls in float32r, ~1e-3 relative error):
  A) fused QKV projection (xT stationary, [Wq|Wk|Wv]^T moving, N=384),
     RMSNorm (free-dim reductions) + RoPE (norm weight folded into host-side
     cos/sin tables, inverse-rms folded into the rope multiplies via
     scalar_tensor_tensor), then PE-transpose Q,K to [hd, t] layout.
  B) attention per head in transposed layout: S^T[s,t] = K^T.T @ Q^T so that
     softmax denominators come out of the AV matmul via a ones-column
     appended to V, and the AV output O^T[hd, t] is directly the lhsT for
     the out-projection. Causality exploited structurally: only lower
     block-triangle computed; diagonal blocks masked with an additive
     -1e32 tile built from the mask input on the host.
  C) AllGather O^T shards -> [2048, 1024], out-projection computes
     outT = Wo_shard @ O^T_full (psum-resident [256, 1024]), host transposes.

build_nc(reps=N) unrolls the whole pipeline N times (including input DMAs and
the collective) for differential wall-clock timing; grading uses reps=1.
"""
import sys
import numpy as np
import ml_dtypes

sys.path.insert(0, "/opt/trn_rl_repo")

import concourse.bass as bass
import concourse.mybir as mybir
import concourse.tile as tile
import concourse.bacc as bacc
from concourse.bass_utils import run_bass_kernel_spmd
from concourse.masks import make_identity
from contextlib import ExitStack

N_CORES = 8
T, D, H, KVH, HD = 1024, 2048, 32, 8, 64
GROUP = H // KVH          # 4 q-heads per kv-head
HQ = H // N_CORES         # 4 q-heads per core
E = HQ * HD               # 256 = per-core q width
QKV = E + HD + HD         # 384 = fused projection width
NT = T // 128             # 8 t-tiles
NK = D // 128             # 16 contraction tiles
EPS = 1e-6
MASK_NEG = -1e32
F32 = mybir.dt.float32
F32R = mybir.dt.float32r
BF16 = mybir.dt.bfloat16
# dtype knobs: A = QKV-projection inputs, E = exp/V (AV matmul),
# O = gathered attention outputs + Wo (out-projection)
A_DT, E_DT, O_DT = F32R, BF16, BF16

# chunking of the valid t-window [128*i, 1024) into PSUM-bank-aligned pieces:
# a single matmul may not cross the 512-col bank boundary of its psum tile

_NC_CACHE = {}


def chunks_for(width):
    out = []
    for a in range(0, width, 512):
        out.append((a, min(width - a, 512)))
    return out


def build_nc(debug=False, reps=1, no_collective=False):
    key = ("nc", debug, reps, no_collective)
    if key in _NC_CACHE:
        return _NC_CACHE[key]
    nc = bacc.Bacc("TRN2", target_bir_lowering=False, debug=False,
                   num_devices=N_CORES)
    mul = mybir.AluOpType.mult
    add = mybir.AluOpType.add

    xT = nc.dram_tensor("xT", [D, T], A_DT, kind="ExternalInput").ap()
    wqkv = nc.dram_tensor("wqkv", [D, QKV], A_DT, kind="ExternalInput").ap()
    wo = nc.dram_tensor("wo", [D, E], O_DT, kind="ExternalInput").ap()
    cos_all = nc.dram_tensor("cos_all", [T, E + HD], F32,
                             kind="ExternalInput").ap()
    sin_all = nc.dram_tensor("sin_all", [T, E + HD], F32,
                             kind="ExternalInput").ap()
    maskneg = nc.dram_tensor("maskneg", [128, NT, 128], F32,
                             kind="ExternalInput").ap()
    outT = nc.dram_tensor("outT", [E, T], F32, kind="ExternalOutput").ap()
    dbg = {}
    if debug:
        for nm, shape in [("d_qkv", [128, NT, QKV]), ("d_roped", [128, NT, E + HD]),
                          ("d_qT", [128, 2, T]), ("d_kT", [128, T]),
                          ("d_v", [128, NT, 72]), ("d_exp00", [128, T]),
                          ("d_exp03", [128, T]),
                          ("d_pot0", [HD + 1, T]), ("d_ot", [E, T]),
                          ("d_rcp", [1, T]), ("d_rep", [HD, T]),
                          ("d_inv", [128, NT, 8])]:
            dbg[nm] = nc.dram_tensor(nm, shape, F32, kind="ExternalOutput").ap()

    with tile.TileContext(nc) as tc:
        with ExitStack() as top:
            persist = top.enter_context(tc.tile_pool(name="persist", bufs=1))
            dram = top.enter_context(tc.tile_pool(name="dram", bufs=1, space="DRAM"))

            # ---- one-time constants ----
            ident = persist.tile([128, 128], F32)
            make_identity(nc, ident[:])
            ones64_f = persist.tile([1, HD], F32)
            nc.gpsimd.memset(ones64_f[:], 1.0)
            ones64 = persist.tile([1, HD], F32R)
            nc.scalar.copy(ones64[:], ones64_f[:])
            eps_t = persist.tile([128, 1], F32)
            nc.gpsimd.memset(eps_t[:], EPS)
            ones8_f = persist.tile([128, NT], F32)
            nc.gpsimd.memset(ones8_f[:], 1.0)

            # ---- persistent (per-rep-rewritten) tiles ----
            qT_all = persist.tile([128, 2, T], F32R, name="qT_all")
            # kT duplicated into partitions 64-127 so lhsT base-partition
            # matches qT_h for odd heads
            kT_all = persist.tile([128, T], F32R, name="kT_all")
            v_all = persist.tile([128, NT, 72], E_DT, name="v_all")
            cs_c = persist.tile([128, NT, E + HD], F32, name="cs_c")
            cs_s = persist.tile([128, NT, E + HD], F32, name="cs_s")
            mneg = persist.tile([128, NT, 128], F32, name="mneg")
            wo_sb = persist.tile([128, NK, E], O_DT, name="wo_sb")
            ot_tiles = [persist.tile([HD, T], O_DT, name=f"otn{h}")
                        for h in range(HQ)]
            for _rep in range(reps):
                # Shared DRAM collective outputs allow only one writer ->
                # fresh per (rep, head); per-head AllGathers overlap the
                # collective with the next head's attention compute
                ot_drams = [dram.tile([HD, T], O_DT, name=f"ot_d{_rep}_{h}",
                                      tag=f"ot_d{_rep}_{h}")
                            for h in range(HQ)]
                ag_drams = [dram.tile([N_CORES * HD, T], O_DT,
                                      name=f"ag_d{_rep}_{h}",
                                      tag=f"ag_d{_rep}_{h}",
                                      addr_space="Shared")
                            for h in range(HQ)]
                # steady-state loads, re-issued per rep so timing counts them
                nc.sync.dma_start(cs_c[:],
                                  cos_all.rearrange("(j p) n -> p j n", p=128))
                nc.sync.dma_start(cs_s[:],
                                  sin_all.rearrange("(j p) n -> p j n", p=128))
                nc.sync.dma_start(mneg[:], maskneg[:])
                nc.sync.dma_start(wo_sb[:], wo.rearrange("(k p) n -> p k n", p=128))
                nc.scalar.copy(v_all[:, :, HD:HD + 1],
                               ones8_f[:].rearrange("p (f o) -> p f o", o=1))

                # ---- phase A: QKV projection + norm + rope + transposes ----
                with ExitStack() as pa:
                    xpool = pa.enter_context(tc.tile_pool(name="xt", bufs=1))
                    scratch = pa.enter_context(tc.tile_pool(name="scrA", bufs=3))
                    ps_qkv = pa.enter_context(
                        tc.tile_pool(name="ps_qkv", bufs=2, space="PSUM"))
                    ps_tr = pa.enter_context(
                        tc.tile_pool(name="ps_tr", bufs=2, space="PSUM"))

                    xt_g = []
                    wq_g = []
                    for g in range(4):
                        xt = xpool.tile([128, 4, T], A_DT, name=f"xt{g}", tag=f"xt{g}")
                        nc.gpsimd.dma_start(
                            xt[:], xT.rearrange("(k p) t -> p k t", p=128)[:, 4 * g:4 * g + 4, :])
                        xt_g.append(xt)
                        wq = xpool.tile([128, 4, QKV], A_DT, name=f"wq{g}", tag=f"wq{g}")
                        nc.gpsimd.dma_start(
                            wq[:], wqkv.rearrange("(k p) n -> p k n", p=128)[:, 4 * g:4 * g + 4, :])
                        wq_g.append(wq)

                    for j in range(NT):
                        pq = ps_qkv.tile([128, QKV], F32, name="pq")
                        for k in range(NK):
                            nc.tensor.matmul(
                                pq[:],
                                xt_g[k // 4][:, k % 4, 128 * j:128 * (j + 1)],
                                wq_g[k // 4][:, k % 4, :],
                                start=(k == 0), stop=(k == NK - 1))
                        # evict V slice (+1s col pre-set), Q|K slice
                        if debug:
                            dq = scratch.tile([128, QKV], F32, name="dq", tag="dq")
                            nc.vector.tensor_copy(dq[:], pq[:])
                            nc.sync.dma_start(dbg["d_qkv"][:, j, :], dq[:])
                        nc.scalar.copy(v_all[:, j, 0:HD], pq[:, E + HD:QKV])
                        qk = scratch.tile([128, E + HD], F32, name="qk", tag="qk")
                        nc.scalar.copy(qk[:], pq[:, 0:E + HD])

                        # rmsnorm stats: square once, segmented reduce
                        sq = scratch.tile([128, E + HD], F32, name="sq", tag="sq")
                        ssq = scratch.tile([128, 8], F32, name="ssq", tag="ssq")
                        nc.vector.tensor_tensor(out=sq[:], in0=qk[:], in1=qk[:],
                                                op=mul)
                        nc.vector.tensor_reduce(
                            out=ssq[:, 0:HQ + 1],
                            in_=sq[:].rearrange("p (h d) -> p h d", h=HQ + 1),
                            axis=mybir.AxisListType.X, op=add)
                        rms = scratch.tile([128, 8], F32, name="rms", tag="rms")
                        nc.scalar.activation(rms[:, 0:HQ + 1], ssq[:, 0:HQ + 1],
                                             mybir.ActivationFunctionType.Sqrt,
                                             scale=1.0 / HD, bias=eps_t[:])
                        inv = scratch.tile([128, 8], F32, name="inv", tag="inv")
                        nc.vector.reciprocal(inv[:, 0:HQ + 1], rms[:, 0:HQ + 1])

                        # rope with inv-rms folded in, merged across head
                        # slices via broadcast / strided views:
                        #   qs = qk * inv[head]; m1 = qs * cos; m2 = swap(qs) * sin
                        qs = scratch.tile([128, E + HD], F32, name="qs", tag="qs")
                        m1 = scratch.tile([128, E + HD], F32, name="m1", tag="m1")
                        m2 = scratch.tile([128, E + HD], F32, name="m2", tag="m2")
                        roped = scratch.tile([128, E + HD], F32, name="roped", tag="roped")
                        hview = lambda ap: ap.rearrange("p (h d) -> p h d", h=HQ + 1)
                        inv_b = (inv[:, 0:HQ + 1]
                                 .rearrange("p (h o) -> p h o", o=1)
                                 .broadcast_to([128, HQ + 1, HD]))
                        nc.vector.tensor_tensor(out=hview(qs[:]), in0=hview(qk[:]),
                                                in1=inv_b, op=mul)
                        nc.vector.tensor_tensor(
                            out=hview(m1[:]), in0=hview(qs[:]),
                            in1=hview(cs_c[:, j, :]), op=mul)
                        nc.vector.tensor_tensor(
                            out=hview(m2[:])[:, :, 0:32],
                            in0=hview(qs[:])[:, :, 32:HD],
                            in1=hview(cs_s[:, j, :])[:, :, 0:32], op=mul)
                        nc.vector.tensor_tensor(
                            out=hview(m2[:])[:, :, 32:HD],
                            in0=hview(qs[:])[:, :, 0:32],
                            in1=hview(cs_s[:, j, :])[:, :, 32:HD], op=mul)
                        nc.vector.tensor_tensor(out=roped[:], in0=m1[:], in1=m2[:], op=add)
                        if debug:
                            nc.sync.dma_start(dbg["d_roped"][:, j, :], roped[:])
                            nc.sync.dma_start(dbg["d_inv"][:, j, :], inv[:])

                        # transpose Q (2 tiles) and K (1) into [hd, t] layout
                        for m in range(2):
                            ptr = ps_tr.tile([128, 128], F32, name="ptr", tag="ptr")
                            nc.tensor.transpose(
                                ptr[:], roped[:, 128 * m:128 * (m + 1)], ident[:])
                            nc.scalar.copy(qT_all[:, m, 128 * j:128 * (j + 1)], ptr[:])
                        ptrk = ps_tr.tile([HD, 128], F32, name="ptrk", tag="ptrk")
                        nc.tensor.transpose(ptrk[:], roped[:, E:E + HD], ident[:])
                        nc.scalar.copy(kT_all[0:HD, 128 * j:128 * (j + 1)], ptrk[:])

                # duplicate kT into partitions 64..127 (cross-partition -> DMA)
                nc.sync.dma_start(kT_all[HD:128, :], kT_all[0:HD, :])
                if debug:
                    nc.sync.dma_start(dbg["d_qT"][:], qT_all[:].bitcast(F32))
                    nc.sync.dma_start(dbg["d_kT"][:], kT_all[:].bitcast(F32))
                    nc.sync.dma_start(dbg["d_v"][:], v_all[:].bitcast(F32))

                # ---- phase B: attention per head, transposed layout ----
                with ExitStack() as pb:
                    epool = pb.enter_context(tc.tile_pool(name="expp", bufs=10))
                    bscr = pb.enter_context(tc.tile_pool(name="scrB", bufs=3))
                    ps_st = pb.enter_context(
                        tc.tile_pool(name="ps_st", bufs=2, space="PSUM"))
                    ps_ot = pb.enter_context(
                        tc.tile_pool(name="ps_ot", bufs=1, space="PSUM"))
                    ps_rep = pb.enter_context(
                        tc.tile_pool(name="ps_rep", bufs=1, space="PSUM"))

                    for h in range(HQ):
                        p0 = 64 * (h % 2)
                        qT_h = qT_all[p0:p0 + 64, h // 2, :]
                        pot = ps_ot.tile([HD + 1, T], F32, name="pot", tag="pot")
                        exp_tiles = []
                        for i in range(NT):
                            t0 = 128 * i
                            w = T - t0
                            et = epool.tile([128, T], E_DT, name=f"et{h}_{i}", tag="et")
                            exp_tiles.append(et)
                            pst = ps_st.tile([128, T], F32, name="pst", tag="pst")
                            for (c0, cw) in chunks_for(w):
                                nc.tensor.matmul(
                                    pst[:, c0:c0 + cw],
                                    kT_all[p0:p0 + HD, t0:t0 + 128],
                                    qT_h[:, t0 + c0:t0 + c0 + cw],
                                    start=True, stop=True)
                            nc.vector.tensor_tensor(
                                out=pst[:, 0:128], in0=pst[:, 0:128],
                                in1=mneg[:, i, :], op=add)
                            nc.scalar.activation(
                                et[:, 0:w], pst[:, 0:w],
                                mybir.ActivationFunctionType.Exp,
                                scale=float(1.0 / np.sqrt(HD)))
                            # AV accumulation for this s-tile
                            for (a, b) in ((0, 512), (512, 1024)):
                                if t0 >= b:
                                    continue
                                lo = max(a, t0)
                                nc.tensor.matmul(
                                    pot[:, lo:b],
                                    v_all[:, i, 0:HD + 1],
                                    et[:, lo - t0:b - t0],
                                    start=(i == 0), stop=(i == min(b // 128, NT) - 1))
                        if debug and h == 0:
                            for (inm, ii) in [("d_exp00", 0), ("d_exp03", 3)]:
                                nc.sync.dma_start(dbg[inm][:], exp_tiles[ii][:].bitcast(F32))
                            dpot = bscr.tile([HD + 1, T], F32, name="dpot", tag="dpot")
                            nc.vector.tensor_copy(dpot[:], pot[:])
                            nc.sync.dma_start(dbg["d_pot0"][:], dpot[:])
                        # softmax denominators -> reciprocal -> replicate -> scale.
                        # the sums row lives at psum partition 64; custom DVE ops
                        # only work at base partition 0, so hop partitions via DMA.
                        sums = bscr.tile([HD + 1, T], F32, name="sums", tag="sums")
                        nc.scalar.copy(sums[HD:HD + 1, :], pot[HD:HD + 1, :])
                        sums0 = bscr.tile([1, T], F32, name="sums0", tag="sums0")
                        nc.sync.dma_start(sums0[:], sums[HD:HD + 1, :])
                        rcp = bscr.tile([1, T], F32, name="rcp", tag="rcp")
                        nc.vector.reciprocal_approx_fast(out=rcp[:], in_=sums0[:])
                        rcp_r = bscr.tile([1, T], F32R, name="rcp_r", tag="rcp_r")
                        nc.scalar.copy(rcp_r[:], rcp[:])
                        prep = ps_rep.tile([HD, T], F32, name="prep", tag="prep")
                        for (a, b) in ((0, 512), (512, 1024)):
                            nc.tensor.matmul(prep[:, a:b], ones64[:],
                                             rcp_r[:, a:b],
                                             start=True, stop=True)
                        rep_sb = bscr.tile([HD, T], F32, name="rep_sb", tag="rep_sb")
                        nc.scalar.copy(rep_sb[:], prep[:])
                        if debug and h == 0:
                            nc.sync.dma_start(dbg["d_rcp"][:], rcp_r[:].bitcast(F32))
                            nc.sync.dma_start(dbg["d_rep"][:], rep_sb[:])
                        nc.vector.tensor_tensor(
                            out=ot_tiles[h][:], in0=pot[0:HD, :], in1=rep_sb[:], op=mul)
                        # launch this head's AllGather immediately so it
                        # overlaps the next head's attention compute
                        nc.sync.dma_start(ot_drams[h][:], ot_tiles[h][:])
                        if debug:
                            nc.sync.dma_start(dbg["d_ot"][HD * h:HD * (h + 1), :],
                                              ot_tiles[h][:].bitcast(F32))
                        if no_collective:
                            nc.sync.dma_start(ag_drams[h][0:HD, :], ot_drams[h][:])
                        else:
                            nc.gpsimd.collective_compute(
                                "AllGather", mybir.AluOpType.bypass,
                                replica_groups=[list(range(N_CORES))],
                                ins=[ot_drams[h].opt()], outs=[ag_drams[h].opt()])

                # ---- phase C: out-projection on the gathered heads ----

                with ExitStack() as pc:
                    agp = pc.enter_context(tc.tile_pool(name="agp", bufs=1))
                    oscr = pc.enter_context(tc.tile_pool(name="oscr", bufs=2))
                    ps_o = pc.enter_context(
                        tc.tile_pool(name="ps_o", bufs=2, space="PSUM"))

                    ag_g = []
                    for g in range(4):
                        agt = agp.tile([128, 4, T], O_DT, name=f"ag{g}", tag=f"ag{g}")
                        nc.sync.dma_start(
                            agt[:],
                            ag_drams[g][:].rearrange("(k p) t -> p k t", p=128))
                        ag_g.append(agt)

                    out_sb = oscr.tile([128, 2, T], F32, name="out_sb", tag="out_sb")
                    for m in range(2):
                        po = ps_o.tile([128, T], F32, name="po", tag="po")
                        for (a, b) in ((0, 512), (512, 1024)):
                            for k in range(NK):
                                nc.tensor.matmul(
                                    po[:, a:b],
                                    wo_sb[:, k, 128 * m:128 * (m + 1)],
                                    ag_g[k // 4][:, k % 4, a:b],
                                    start=(k == 0), stop=(k == NK - 1))
                        if m == 0:
                            nc.scalar.copy(out_sb[:, m, :], po[:])
                        else:
                            nc.vector.tensor_copy(out_sb[:, m, :], po[:])
                    nc.sync.dma_start(
                        outT[:].rearrange("(f p) t -> p f t", p=128), out_sb[:])

    nc.compile()
    _NC_CACHE[key] = nc
    return nc


def prep_in_maps(inputs):
    """Host-side sharding + layout prep. Returns per-core input maps."""
    x = np.asarray(inputs["x"], dtype=np.float32)
    mask = np.asarray(inputs["mask"])
    cos = np.asarray(inputs["cos"], dtype=np.float32)
    sin = np.asarray(inputs["sin"], dtype=np.float32)
    Wq = np.asarray(inputs["Wq"], dtype=np.float32)
    Wk = np.asarray(inputs["Wk"], dtype=np.float32)
    Wv = np.asarray(inputs["Wv"], dtype=np.float32)
    Wo = np.asarray(inputs["Wo"], dtype=np.float32)
    qw = np.asarray(inputs["q_norm_w"], dtype=np.float32)
    kw = np.asarray(inputs["k_norm_w"], dtype=np.float32)

    xT = np.ascontiguousarray(x.T)

    # norm weights folded into per-(q/k) rope tables; sin table carries the
    # rotate-half signs: out[i] = w[i]*(q[i]*cos[i] + rot[i]*sin[i]),
    # rot[i] = -q[i+32] (i<32) else q[i-32]
    sgn = np.concatenate([-np.ones(HD // 2, np.float32),
                          np.ones(HD // 2, np.float32)])
    cos_q = cos * qw[None, :]
    sin_q = sin * (sgn * qw)[None, :]
    cos_k = cos * kw[None, :]
    sin_k = sin * (sgn * kw)[None, :]
    # combined per-slice tables: 4x q-head then 1x k-head, [T, 320]
    cos_all = np.ascontiguousarray(
        np.concatenate([cos_q] * HQ + [cos_k], axis=1))
    sin_all = np.ascontiguousarray(
        np.concatenate([sin_q] * HQ + [sin_k], axis=1))

    # additive mask for diagonal blocks, in S^T layout:
    # maskneg[s', i, t'] = MASK_NEG where mask[128i+t', 128i+s']
    mneg = np.zeros((128, NT, 128), np.float32)
    for i in range(NT):
        blk = mask[128 * i:128 * (i + 1), 128 * i:128 * (i + 1)]
        mneg[:, i, :] = np.where(blk.T, MASK_NEG, 0.0).astype(np.float32)

    def cast_a(v):
        return v.astype(ml_dtypes.bfloat16) if A_DT == BF16 else v

    def cast_o(v):
        return v.astype(ml_dtypes.bfloat16) if O_DT == BF16 else v

    xT = cast_a(xT)
    in_maps = []
    for r in range(N_CORES):
        wq_r = Wq[E * r:E * (r + 1), :]          # [256, 2048]
        wk_r = Wk[HD * r:HD * (r + 1), :]        # [64, 2048]
        wv_r = Wv[HD * r:HD * (r + 1), :]        # [64, 2048]
        wqkv = np.ascontiguousarray(
            np.concatenate([wq_r, wk_r, wv_r], axis=0).T)   # [2048, 384]
        wo_rT = Wo[E * r:E * (r + 1), :].T                      # [2048, 256]
        # reorder contraction rows to match the per-head AllGather layout:
        # gathered buffer h holds rows (64*rank + d) = global hd 256*rank+64h+d
        wo_re = np.empty_like(wo_rT)
        for h in range(HQ):
            for rr in range(N_CORES):
                wo_re[512 * h + 64 * rr:512 * h + 64 * rr + 64] = \
                    wo_rT[E * rr + HD * h:E * rr + HD * h + 64]
        in_maps.append({
            "xT": xT, "wqkv": cast_a(wqkv),
            "wo": cast_o(np.ascontiguousarray(wo_re)),
            "cos_all": cos_all, "sin_all": sin_all,
            "maskneg": mneg,
        })
    return in_maps


def kernel(**inputs) -> np.ndarray:
    nc = build_nc()
    in_maps = prep_in_maps(inputs)
    res = run_bass_kernel_spmd(nc, in_maps, list(range(N_CORES)))
    out = np.empty((T, D), dtype=np.float32)
    for r in range(N_CORES):
        out[:, E * r:E * (r + 1)] = res.results[r]["outT"].T
    return out

